# revision 3
# baseline (speedup 1.0000x reference)
"""Trainium2 Bass kernel for nn_Conv2d (B=32, 256->256, 56x56, 3x3, pad=1) + bias.

Strategy
--------
Data-parallel over batch: 4 images per NeuronCore x 8 cores; weights/bias
replicated; no collectives.

Per core, the conv is computed as shifted matmuls on the 58-wide zero-padded
input (output position (h,w) <-> flat index h*58+w, tap (kh,kw) contribution is
a matmul against the input shifted by kh*58+kw).

The matmuls run in fp8-e4m3 DoubleRow mode (0.5 cycles/output element, 256-wide
contraction per matmul: the two cin chunks ride the DR pair dimension), which
is 4x the f32r MAC rate.  e4m3 alone is too coarse (measured 3.1e-2 rel err vs
the 2e-2 gate), so residual-correction planes are added:

  X16  = q(16*x)          W512 = q(512*w)
  Xlo  = q(16*x - X16)    Wlo  = q(512*w - W512)

  out*8192 ~= conv(X16 + Xlo, W512) + conv(X16, Wlo restricted to 5 of 9 taps)

i.e. per output tile: 9 main DR matmuls + 9 x-residual DR matmuls (sharing the
main stationary weights) + 5 w-residual DR matmuls = 23, vs 36 f32r-equivalents
-> 1.57x less PE streaming.  Measured rel err vs the f32 reference: 1.51e-2
(w-residual on all 9 taps gives 9.0e-4 at 27 matmuls, the safe fallback).
All quantized values sit in e4m3's normal range so PE denormal handling is
immaterial (verified numerically both ways).  PSUM accumulates in f32 at scale
2^13; bias is pre-scaled by 2^13 on the host and the final 2^-13 is applied on
the host after gathering (exact, power of two).
"""

import numpy as np
import ml_dtypes

import concourse.bacc as bacc
import concourse.tile as tile
import concourse.mybir as mybir
from concourse.bass_utils import run_bass_kernel_spmd

F32 = mybir.dt.float32
F8 = mybir.dt.float8e4
E4NP = ml_dtypes.float8_e4m3
DR = mybir.MatmulPerfMode.DoubleRow

B, CIN, COUT, H, W, K = 32, 256, 256, 56, 56, 3
NCORES = 8
BPC = B // NCORES          # images per core
WP = W + 2                 # padded row width (58)
HP = H + 3                 # padded rows (59): 1 top, 2 bottom (tail tap reads)
XF = HP * WP               # padded flat length per image-channel (3422)
OF = H * WP                # padded output flat length (3248)
NT = 7                     # output tiles per (img, cout-chunk)
NFREE = OF // NT           # 464 positions per matmul
XLOAD = 3366               # matmuls never read past 3365
SCALE = 8192.0             # 16 * 512 product scale, undone on host

WCORR_TAPS = (0, 2, 4, 6, 8)
# op list per psum tile: (weight_pair_index, x_plane_base, tap)
OPS = []
for t in range(K * K):
    OPS.append((t, 0, t))      # main: X16 pair x W512[t]
    OPS.append((t, 2, t))      # x-residual: Xlo pair x W512[t]
for i, t in enumerate(WCORR_TAPS):
    OPS.append((K * K + i, 0, t))  # w-residual: X16 pair x Wlo[t]
NOPS = len(OPS)            # 23
NPAIR = K * K + len(WCORR_TAPS)  # 14 stationary pairs per cout chunk

_CACHE = {}


def _build():
    if "nc" in _CACHE:
        return _CACHE["nc"]
    nc = bacc.Bacc("TRN2", target_bir_lowering=False, debug=False,
                   num_swdge_queues=4)
    x_d = nc.dram_tensor("x", [BPC, 128, 4, XF], F8, kind="ExternalInput").ap()
    w_d = nc.dram_tensor("w", [2, 128, NPAIR, 2, 128], F8,
                         kind="ExternalInput").ap()
    b_d = nc.dram_tensor("b", [COUT], F32, kind="ExternalInput").ap()
    o_d = nc.dram_tensor("o", [BPC, COUT, OF], F32, kind="ExternalOutput").ap()

    with tile.TileContext(nc) as tc:
        with (
            tc.tile_pool(name="wp", bufs=1) as wp,
            tc.tile_pool(name="xp", bufs=3) as xp,
            tc.tile_pool(name="op", bufs=2) as op,
            tc.tile_pool(name="pp", bufs=8, space="PSUM") as pp,
        ):
            eng = [nc.sync, nc.scalar]

            bias_t = wp.tile([128, 2], F32)
            w_t = wp.tile([128, 2, NPAIR, 2, 128], F8)

            # weights + bias via SWDGE so sync/scalar stay free for x slices
            for cc in range(2):
                nc.gpsimd.dma_start(out=w_t[:, cc], in_=w_d[cc])
                nc.gpsimd.dma_start(out=bias_t[:, cc:cc + 1],
                                    in_=b_d[cc * 128:(cc + 1) * 128])

            # x slice boundaries: every run >= 512B to dodge the small-
            # descriptor DMA penalty; early slices unblock wave 1 (needs
            # [0, 1974)) as they land
            XSL = [0, 582, 1160, 1742, 2324, 2906, XLOAD]

            def load_img(img):
                x_t = xp.tile([128, 4, XF], F8, tag="x")
                for s in range(len(XSL) - 1):
                    eng[s % 2].dma_start(
                        out=x_t[:, :, XSL[s]:XSL[s + 1]],
                        in_=x_d[img, :, :, XSL[s]:XSL[s + 1]],
                    )
                return x_t

            WAVES = [[0, 1, 2, 3], [4, 5, 6]]

            for img in range(BPC):
                x_t = load_img(img)
                for cc in range(2):
                    o_t = op.tile([128, OF], F32, tag="o")
                    for wv, wave in enumerate(WAVES):
                        pss = [pp.tile([128, NFREE], F32, tag="ps",
                                       name=f"ps{nt}")
                               for nt in wave]
                        for oi, (pair, pb, tap) in enumerate(OPS):
                            dlt = (tap // K) * WP + tap % K
                            for ps, nt in zip(pss, wave):
                                off = nt * NFREE + dlt
                                nc.tensor.matmul(
                                    ps,
                                    w_t[:, cc, pair],
                                    x_t[:, pb:pb + 2, off:off + NFREE],
                                    start=(oi == 0),
                                    stop=(oi == NOPS - 1),
                                    perf_mode=DR,
                                )
                        for ps, nt in zip(pss, wave):
                            # bias-add (pre-scaled 2^13) + PSUM eviction on
                            # the otherwise-idle DVE
                            nc.vector.tensor_scalar_add(
                                o_t[:, nt * NFREE:(nt + 1) * NFREE],
                                ps,
                                bias_t[:, cc:cc + 1],
                            )
                        lo = wave[0] * NFREE
                        hi = (wave[-1] + 1) * NFREE
                        eng[(img + cc + wv) % 2].dma_start(
                            out=o_d[img, cc * 128:(cc + 1) * 128, lo:hi],
                            in_=o_t[:, lo:hi],
                        )
    nc.compile()
    _CACHE["nc"] = nc
    return nc


def make_in_maps(inp, kernel, bias):
    xpad = np.zeros((B, CIN, HP, WP), np.float32)
    xpad[:, :, 1:1 + H, 1:1 + W] = inp
    x16 = (16.0 * xpad).reshape(B, 2, 128, XF)
    X16 = x16.astype(E4NP)
    Xlo = (x16 - X16.astype(np.float32)).astype(E4NP)
    xpk = np.empty((B, 128, 4, XF), E4NP)
    xpk[:, :, 0] = X16[:, 0]
    xpk[:, :, 1] = X16[:, 1]
    xpk[:, :, 2] = Xlo[:, 0]
    xpk[:, :, 3] = Xlo[:, 1]

    # [cout, cin, kh, kw] -> [tap, ci, p, cc, cout-in-chunk]
    w512 = (512.0 * np.asarray(kernel, np.float32)).transpose(2, 3, 1, 0) \
        .reshape(K * K, 2, 128, 2, 128)
    W512 = w512.astype(E4NP)
    Wlo = (w512 - W512.astype(np.float32)).astype(E4NP)
    wpk = np.empty((2, 128, NPAIR, 2, 128), E4NP)
    for cc in range(2):
        for t in range(K * K):
            for ci in range(2):
                wpk[cc, :, t, ci] = W512[t, ci, :, cc]
        for i, t in enumerate(WCORR_TAPS):
            for ci in range(2):
                wpk[cc, :, K * K + i, ci] = Wlo[t, ci, :, cc]

    b_dev = np.ascontiguousarray(np.asarray(bias, np.float32) * SCALE)
    return [
        {"x": np.ascontiguousarray(xpk[c * BPC:(c + 1) * BPC]),
         "w": wpk, "b": b_dev}
        for c in range(NCORES)
    ]


def assemble(results):
    o = np.concatenate([results[c]["o"] for c in range(NCORES)], axis=0)
    o *= np.float32(1.0 / SCALE)
    return np.ascontiguousarray(
        o.reshape(B, COUT, H, WP)[:, :, :, :W].astype(np.float32)
    )


def kernel(inp, kernel, bias):
    nc = _build()
    in_maps = make_in_maps(inp, kernel, bias)
    r = run_bass_kernel_spmd(nc, in_maps, core_ids=list(range(NCORES)))
    return assemble(r.results)


# revision 8
# speedup vs baseline: 1.0022x; 1.0022x over previous
"""Trainium2 Bass kernel for nn_Conv2d (B=32, 256->256, 56x56, 3x3, pad=1) + bias.

Strategy
--------
Data-parallel over batch: 4 images per NeuronCore x 8 cores; weights/bias
replicated; no collectives.

Per core, the conv is computed as shifted matmuls on the 58-wide zero-padded
input (output position (h,w) <-> flat index h*58+w, tap (kh,kw) contribution is
a matmul against the input shifted by kh*58+kw).

The matmuls run in fp8-e4m3 DoubleRow mode (0.5 cycles/output element, 256-wide
contraction per matmul: the two cin chunks ride the DR pair dimension), which
is 4x the f32r MAC rate.  e4m3 alone is too coarse (measured 3.1e-2 rel err vs
the 2e-2 gate), so residual-correction planes are added:

  X16  = q(16*x)          W512 = q(512*w)
  Xlo  = q(16*x - X16)    Wlo  = q(512*w - W512)

  out*8192 ~= conv(X16 + Xlo, W512) + conv(X16, Wlo restricted to 5 of 9 taps)

i.e. per output tile: 9 main DR matmuls + 9 x-residual DR matmuls (sharing the
main stationary weights) + 5 w-residual DR matmuls = 23, vs 36 f32r-equivalents
-> 1.57x less PE streaming.  Measured rel err vs the f32 reference: 1.51e-2
(w-residual on all 9 taps gives 9.0e-4 at 27 matmuls, the safe fallback).
All quantized values sit in e4m3's normal range so PE denormal handling is
immaterial (verified numerically both ways).  PSUM accumulates in f32 at scale
2^13; bias is pre-scaled by 2^13 on the host and the final 2^-13 is applied on
the host after gathering (exact, power of two).
"""

import numpy as np
import ml_dtypes

import concourse.bacc as bacc
import concourse.tile as tile
import concourse.mybir as mybir
from concourse.bass_utils import run_bass_kernel_spmd

F32 = mybir.dt.float32
F8 = mybir.dt.float8e4
E4NP = ml_dtypes.float8_e4m3
DR = mybir.MatmulPerfMode.DoubleRow

B, CIN, COUT, H, W, K = 32, 256, 256, 56, 56, 3
NCORES = 8
BPC = B // NCORES          # images per core
WP = W + 2                 # padded row width (58)
HP = H + 3                 # padded rows (59): 1 top, 2 bottom (tail tap reads)
XF = HP * WP               # padded flat length per image-channel (3422)
OF = H * WP                # padded output flat length (3248)
NT = 7                     # output tiles per (img, cout-chunk)
NFREE = OF // NT           # 464 positions per matmul
XLOAD = 3366               # matmuls never read past 3365
SCALE = 8192.0             # 16 * 512 product scale, undone on host

WCORR_TAPS = (0, 2, 4, 6, 8)
# op list per psum tile: (weight_pair_index, x_group, tap)
OPS = []
for t in range(K * K):
    OPS.append((t, 0, t))      # main: X16 pair x W512[t]
    OPS.append((t, 1, t))      # x-residual: Xlo pair x W512[t]
for i, t in enumerate(WCORR_TAPS):
    OPS.append((K * K + i, 0, t))  # w-residual: X16 pair x Wlo[t]
NOPS = len(OPS)            # 23
NPAIR = K * K + len(WCORR_TAPS)  # 14 stationary pairs per cout chunk

_CACHE = {}


def _build():
    if "nc" in _CACHE:
        return _CACHE["nc"]
    nc = bacc.Bacc("TRN2", target_bir_lowering=False, debug=False,
                   num_swdge_queues=4)
    x_d = nc.dram_tensor("x", [BPC, 128, 2, XF, 2], F8,
                         kind="ExternalInput").ap()
    w_d = nc.dram_tensor("w", [2, 128, NPAIR, 2, 128], F8,
                         kind="ExternalInput").ap()
    b_d = nc.dram_tensor("b", [COUT], F32, kind="ExternalInput").ap()
    o_d = nc.dram_tensor("o", [BPC, COUT, OF], F32, kind="ExternalOutput").ap()

    with tile.TileContext(nc) as tc:
        with (
            tc.tile_pool(name="wp", bufs=1) as wp,
            tc.tile_pool(name="xp", bufs=3) as xp,
            tc.tile_pool(name="op", bufs=2) as op,
            tc.tile_pool(name="pp", bufs=8, space="PSUM") as pp,
        ):
            eng = [nc.sync, nc.scalar]

            bias_t = wp.tile([128, 2], F32)
            w_t = wp.tile([128, 2, NPAIR, 2, 128], F8)

            # weights + bias via SWDGE so sync/scalar stay free for x slices
            for cc in range(2):
                nc.gpsimd.dma_start(out=w_t[:, cc], in_=w_d[cc])
                nc.gpsimd.dma_start(out=bias_t[:, cc:cc + 1],
                                    in_=b_d[cc * 128:(cc + 1) * 128])

            # x slice boundaries: every run >= 512B to dodge the small-
            # descriptor DMA penalty; early slices unblock wave 1 (needs
            # [0, 1974)) as they land
            XSL = [0, 582, 1160, 1742, 2324, 2906, XLOAD]

            def load_img(img):
                # pair-interleaved layout: per partition, group g holds
                # (col, ci) pairs adjacent so the PE streams one aligned
                # 2-byte pair per partition per cycle in DoubleRow mode
                x_t = xp.tile([128, 2, XF, 2], F8, tag="x")
                for s in range(len(XSL) - 1):
                    eng[s % 2].dma_start(
                        out=x_t[:, :, XSL[s]:XSL[s + 1], :],
                        in_=x_d[img, :, :, XSL[s]:XSL[s + 1], :],
                    )
                return x_t

            WAVES = [[0, 1, 2, 3], [4, 5, 6]]

            for img in range(BPC):
                x_t = load_img(img)
                for cc in range(2):
                    o_t = op.tile([128, OF], F32, tag="o")
                    for wv, wave in enumerate(WAVES):
                        pss = [pp.tile([128, NFREE], F32, tag="ps",
                                       name=f"ps{nt}")
                               for nt in wave]
                        for oi, (pair, pb, tap) in enumerate(OPS):
                            dlt = (tap // K) * WP + tap % K
                            for ps, nt in zip(pss, wave):
                                off = nt * NFREE + dlt
                                nc.tensor.matmul(
                                    ps,
                                    w_t[:, cc, pair],
                                    x_t[:, pb, off:off + NFREE, :]
                                        .transpose([0, 2, 1]),
                                    start=(oi == 0),
                                    stop=(oi == NOPS - 1),
                                    perf_mode=DR,
                                )
                        for ps, nt in zip(pss, wave):
                            # bias-add (pre-scaled 2^13) + PSUM eviction on
                            # the otherwise-idle DVE
                            nc.vector.tensor_scalar_add(
                                o_t[:, nt * NFREE:(nt + 1) * NFREE],
                                ps,
                                bias_t[:, cc:cc + 1],
                            )
                        lo = wave[0] * NFREE
                        hi = (wave[-1] + 1) * NFREE
                        eng[(img + cc + wv) % 2].dma_start(
                            out=o_d[img, cc * 128:(cc + 1) * 128, lo:hi],
                            in_=o_t[:, lo:hi],
                        )
    nc.compile()
    _CACHE["nc"] = nc
    return nc


def make_in_maps(inp, kernel, bias):
    xpad = np.zeros((B, CIN, HP, WP), np.float32)
    xpad[:, :, 1:1 + H, 1:1 + W] = inp
    x16 = (16.0 * xpad).reshape(B, 2, 128, XF)
    X16 = x16.astype(E4NP)
    Xlo = (x16 - X16.astype(np.float32)).astype(E4NP)
    xpk = np.empty((B, 128, 2, XF, 2), E4NP)
    xpk[:, :, 0, :, 0] = X16[:, 0]
    xpk[:, :, 0, :, 1] = X16[:, 1]
    xpk[:, :, 1, :, 0] = Xlo[:, 0]
    xpk[:, :, 1, :, 1] = Xlo[:, 1]

    # [cout, cin, kh, kw] -> [tap, ci, p, cc, cout-in-chunk]
    w512 = (512.0 * np.asarray(kernel, np.float32)).transpose(2, 3, 1, 0) \
        .reshape(K * K, 2, 128, 2, 128)
    W512 = w512.astype(E4NP)
    Wlo = (w512 - W512.astype(np.float32)).astype(E4NP)
    wpk = np.empty((2, 128, NPAIR, 2, 128), E4NP)
    for cc in range(2):
        for t in range(K * K):
            for ci in range(2):
                wpk[cc, :, t, ci] = W512[t, ci, :, cc]
        for i, t in enumerate(WCORR_TAPS):
            for ci in range(2):
                wpk[cc, :, K * K + i, ci] = Wlo[t, ci, :, cc]

    b_dev = np.ascontiguousarray(np.asarray(bias, np.float32) * SCALE)
    return [
        {"x": np.ascontiguousarray(xpk[c * BPC:(c + 1) * BPC]),
         "w": wpk, "b": b_dev}
        for c in range(NCORES)
    ]


def assemble(results):
    o = np.concatenate([results[c]["o"] for c in range(NCORES)], axis=0)
    o *= np.float32(1.0 / SCALE)
    return np.ascontiguousarray(
        o.reshape(B, COUT, H, WP)[:, :, :, :W].astype(np.float32)
    )


def kernel(inp, kernel, bias):
    nc = _build()
    in_maps = make_in_maps(inp, kernel, bias)
    r = run_bass_kernel_spmd(nc, in_maps, core_ids=list(range(NCORES)))
    return assemble(r.results)


# revision 12
# speedup vs baseline: 1.0036x; 1.0014x over previous
"""Trainium2 Bass kernel for nn_Conv2d (B=32, 256->256, 56x56, 3x3, pad=1) + bias.

Strategy
--------
Data-parallel over batch: 4 images per NeuronCore x 8 cores; weights/bias
replicated; no collectives.

Per core, the conv is computed as shifted matmuls on the 58-wide zero-padded
input (output position (h,w) <-> flat index h*58+w, tap (kh,kw) contribution is
a matmul against the input shifted by kh*58+kw).

The matmuls run in fp8-e4m3 DoubleRow mode (0.5 cycles/output element, 256-wide
contraction per matmul: the two cin chunks ride the DR pair dimension), which
is 4x the f32r MAC rate.  e4m3 alone is too coarse (measured 3.1e-2 rel err vs
the 2e-2 gate), so residual-correction planes are added:

  X16  = q(16*x)          W512 = q(512*w)
  Xlo  = q(16*x - X16)    Wlo  = q(512*w - W512)

  out*8192 ~= conv(X16 + Xlo, W512) + conv(X16, Wlo restricted to 5 of 9 taps)

i.e. per output tile: 9 main DR matmuls + 9 x-residual DR matmuls (sharing the
main stationary weights) + 5 w-residual DR matmuls = 23, vs 36 f32r-equivalents
-> 1.57x less PE streaming.  Measured rel err vs the f32 reference: 1.51e-2
(w-residual on all 9 taps gives 9.0e-4 at 27 matmuls, the safe fallback).
All quantized values sit in e4m3's normal range so PE denormal handling is
immaterial (verified numerically both ways).  PSUM accumulates in f32 at scale
2^13; bias is pre-scaled by 2^13 on the host and the final 2^-13 is applied on
the host after gathering (exact, power of two).
"""

import numpy as np
import ml_dtypes

import concourse.bacc as bacc
import concourse.tile as tile
import concourse.mybir as mybir
from concourse.bass_utils import run_bass_kernel_spmd

F32 = mybir.dt.float32
F8 = mybir.dt.float8e4
E4NP = ml_dtypes.float8_e4m3
DR = mybir.MatmulPerfMode.DoubleRowSwInterleave

B, CIN, COUT, H, W, K = 32, 256, 256, 56, 56, 3
NCORES = 8
BPC = B // NCORES          # images per core
WP = W + 2                 # padded row width (58)
HP = H + 3                 # padded rows (59): 1 top, 2 bottom (tail tap reads)
XF = HP * WP               # padded flat length per image-channel (3422)
OF = H * WP                # padded output flat length (3248)
NT = 7                     # output tiles per (img, cout-chunk)
NFREE = OF // NT           # 464 positions per matmul
XLOAD = 3366               # matmuls never read past 3365
SCALE = 8192.0             # 16 * 512 product scale, undone on host

WCORR_TAPS = (0, 2, 4, 6, 8)
# op list per psum tile: (weight_pair_index, x_group, tap)
OPS = []
for t in range(K * K):
    OPS.append((t, 0, t))      # main: X16 pair x W512[t]
    OPS.append((t, 1, t))      # x-residual: Xlo pair x W512[t]
for i, t in enumerate(WCORR_TAPS):
    OPS.append((K * K + i, 0, t))  # w-residual: X16 pair x Wlo[t]
NOPS = len(OPS)            # 23
NPAIR = K * K + len(WCORR_TAPS)  # 14 stationary pairs per cout chunk

_CACHE = {}


def _build():
    if "nc" in _CACHE:
        return _CACHE["nc"]
    nc = bacc.Bacc("TRN2", target_bir_lowering=False, debug=False,
                   num_swdge_queues=4)
    x_d = nc.dram_tensor("x", [BPC, 128, 2, XF, 2], F8,
                         kind="ExternalInput").ap()
    w_d = nc.dram_tensor("w", [2, 128, NPAIR, 256], F8,
                         kind="ExternalInput").ap()
    b_d = nc.dram_tensor("b", [COUT], F32, kind="ExternalInput").ap()
    o_d = nc.dram_tensor("o", [BPC, COUT, OF], F32, kind="ExternalOutput").ap()

    with tile.TileContext(nc) as tc:
        with (
            tc.tile_pool(name="wp", bufs=1) as wp,
            tc.tile_pool(name="xp", bufs=3) as xp,
            tc.tile_pool(name="op", bufs=2) as op,
            tc.tile_pool(name="pp", bufs=8, space="PSUM") as pp,
        ):
            eng = [nc.sync, nc.scalar]

            bias_t = wp.tile([128, 2], F32)
            # SwInterleave stationary layout: per cin partition,
            # (ci0, ci1) pairs interleaved per cout column, couts reversed
            w_t = wp.tile([128, 2, NPAIR, 256], F8)

            # weights + bias via SWDGE so sync/scalar stay free for x slices
            for cc in range(2):
                nc.gpsimd.dma_start(out=w_t[:, cc], in_=w_d[cc])
                nc.gpsimd.dma_start(out=bias_t[:, cc:cc + 1],
                                    in_=b_d[cc * 128:(cc + 1) * 128])

            # x slice boundaries: every run >= 512B to dodge the small-
            # descriptor DMA penalty; early slices unblock wave 1 (needs
            # [0, 1974)) as they land
            XSL = [0, 582, 1160, 1742, 2324, 2906, XLOAD]

            def load_img(img):
                # pair-interleaved layout: per partition, group g holds
                # (col, ci) pairs adjacent so the PE streams one aligned
                # 2-byte pair per partition per cycle in DoubleRow mode
                x_t = xp.tile([128, 2, XF, 2], F8, tag="x")
                for s in range(len(XSL) - 1):
                    eng[s % 2].dma_start(
                        out=x_t[:, :, XSL[s]:XSL[s + 1], :],
                        in_=x_d[img, :, :, XSL[s]:XSL[s + 1], :],
                    )
                return x_t

            WAVES = [[0, 1, 2, 3], [4, 5, 6]]

            for img in range(BPC):
                x_t = load_img(img)
                for cc in range(2):
                    o_t = op.tile([128, OF], F32, tag="o")
                    for wv, wave in enumerate(WAVES):
                        pss = [pp.tile([128, NFREE], F32, tag="ps",
                                       name=f"ps{nt}")
                               for nt in wave]
                        for oi, (pair, pb, tap) in enumerate(OPS):
                            dlt = (tap // K) * WP + tap % K
                            for ps, nt in zip(pss, wave):
                                off = nt * NFREE + dlt
                                nc.tensor.matmul(
                                    ps,
                                    w_t[:, cc, pair],
                                    x_t[:, pb, off:off + NFREE, :]
                                        .transpose([0, 2, 1]),
                                    start=(oi == 0),
                                    stop=(oi == NOPS - 1),
                                    perf_mode=DR,
                                )
                        for ps, nt in zip(pss, wave):
                            # bias-add (pre-scaled 2^13) + PSUM eviction on
                            # the otherwise-idle DVE
                            nc.vector.tensor_scalar_add(
                                o_t[:, nt * NFREE:(nt + 1) * NFREE],
                                ps,
                                bias_t[:, cc:cc + 1],
                            )
                        lo = wave[0] * NFREE
                        hi = (wave[-1] + 1) * NFREE
                        eng[(img + cc + wv) % 2].dma_start(
                            out=o_d[img, cc * 128:(cc + 1) * 128, lo:hi],
                            in_=o_t[:, lo:hi],
                        )
    nc.compile()
    _CACHE["nc"] = nc
    return nc


def make_in_maps(inp, kernel, bias):
    xpad = np.zeros((B, CIN, HP, WP), np.float32)
    xpad[:, :, 1:1 + H, 1:1 + W] = inp
    x16 = (16.0 * xpad).reshape(B, 2, 128, XF)
    X16 = x16.astype(E4NP)
    Xlo = (x16 - X16.astype(np.float32)).astype(E4NP)
    xpk = np.empty((B, 128, 2, XF, 2), E4NP)
    xpk[:, :, 0, :, 0] = X16[:, 0]
    xpk[:, :, 0, :, 1] = X16[:, 1]
    xpk[:, :, 1, :, 0] = Xlo[:, 0]
    xpk[:, :, 1, :, 1] = Xlo[:, 1]

    # [cout, cin, kh, kw] -> [tap, ci, p, cc, cout-in-chunk]
    w512 = (512.0 * np.asarray(kernel, np.float32)).transpose(2, 3, 1, 0) \
        .reshape(K * K, 2, 128, 2, 128)
    W512 = w512.astype(E4NP)
    Wlo = (w512 - W512.astype(np.float32)).astype(E4NP)
    # SwInterleave weight layout: stored[2j] = ci0[cout 127-j],
    # stored[2j+1] = ci1[cout 127-j]
    wpk = np.empty((2, 128, NPAIR, 256), E4NP)
    for cc in range(2):
        for t in range(K * K):
            wpk[cc, :, t, 0::2] = W512[t, 0, :, cc, ::-1]
            wpk[cc, :, t, 1::2] = W512[t, 1, :, cc, ::-1]
        for i, t in enumerate(WCORR_TAPS):
            wpk[cc, :, K * K + i, 0::2] = Wlo[t, 0, :, cc, ::-1]
            wpk[cc, :, K * K + i, 1::2] = Wlo[t, 1, :, cc, ::-1]

    b_dev = np.ascontiguousarray(np.asarray(bias, np.float32) * SCALE)
    return [
        {"x": np.ascontiguousarray(xpk[c * BPC:(c + 1) * BPC]),
         "w": wpk, "b": b_dev}
        for c in range(NCORES)
    ]


def assemble(results):
    o = np.concatenate([results[c]["o"] for c in range(NCORES)], axis=0)
    o *= np.float32(1.0 / SCALE)
    return np.ascontiguousarray(
        o.reshape(B, COUT, H, WP)[:, :, :, :W].astype(np.float32)
    )


def kernel(inp, kernel, bias):
    nc = _build()
    in_maps = make_in_maps(inp, kernel, bias)
    r = run_bass_kernel_spmd(nc, in_maps, core_ids=list(range(NCORES)))
    return assemble(r.results)
